# revision 2
# baseline (speedup 1.0000x reference)
"""Multi-head attention Trainium2 Bass kernel, 8-way SPMD. v10.

Problem: nn_MultiHeadAttention (B=2, S=4096, D=512, H=8, Dk=64), fp32 I/O.

Sharding: the 8192 (B*S) query rows are split into 8 shards of 1024 rows,
one per NeuronCore (core c takes batch c//4, rows (c%4)*1024..). Each core
holds the full key/value sequence of its batch, so there are no
collectives; the per-core output rows concatenate into the full output.

The kernel is ScalarE-bound: exp of 33.5M scores per core at 1
elem/lane/cycle (plus ~352-cycle per-ACTIVATE overhead) is ~294us.
Everything else hides under a gapless 1-kt exp cadence:
  - all matmul operands bf16; PE performs no transposes. Tensors land
    feature-major via hardware DMA-XBAR transposes (one per 512-row
    chunk) into (row-tile, feature-tile) tiled layouts [128, RT, FC, 128].
  - xk/xv chunks are DMA'd with a partition-contiguous pattern
    ("(p rt) d"): 128 descriptors instead of 512, 4x cheaper on the sync
    queue. This permutes kv rows within each chunk, which is harmless:
    the same permutation lands in both the KT columns and the V rows, and
    softmax+AV contract over the permuted axis.
  - casts fp32->bf16: xv on the vector engine, xk on the otherwise-idle
    GpSimd engine, one op per chunk.
  - pair-major loop; QT[p] is projected in pair p's preseed (through the
    production PSUM pool), so the prologue issues only DMAs and casts and
    the first exp fires as soon as chunk 0 and wq/xq land.
  - PSUM per half: scores dual [128,2,512] (4 banks) + acc (2) +
    production/projection (2) = 8; acc pools alternate per half
    (accA/accB) and are drained by one bf16 stash copy per head, with
    reciprocal+broadcast+multiply deferred a full half (late normalize).
  - out-projection tail: qh0 first (its attT is ready), then pair-3-qh1's
    late normalize, then qh1.

repeat>1 builds unroll the whole kernel for delta-timing; timing=True
builds replace the x inputs with device-initialized Internal DRAM so the
per-call staging overhead (~178MB over the axon tunnel) disappears from
measurements.
"""

from contextlib import ExitStack

import numpy as np

B = 2
S = 4096
D = 512
H = 8
DK = 64
P = 128
N_CORES = 8
SQ = (B * S) // N_CORES  # 1024 query rows per core
SKV = S  # 4096 kv rows per core
FC = D // P  # 4 feature chunks
NKT = SKV // P  # 32 key tiles
NSC = SKV // 512  # 8 seq chunks
QH = SQ // 512  # 2 query halves
INV_SCALE = 0.125  # 1/sqrt(DK)

_CACHE = {}


def _build_nc(repeat: int = 1, timing: bool = False, loop: int = 1):
    import concourse.mybir as mybir
    import concourse.tile as tile
    from concourse import bacc

    f32 = mybir.dt.float32
    bf16 = mybir.dt.bfloat16
    EXP = mybir.ActivationFunctionType.Exp

    nc = bacc.Bacc(
        "TRN2",
        target_bir_lowering=False,
        debug=False,
        enable_asserts=False,
        num_devices=N_CORES,
    )

    def din(name, shape):
        kind = "Internal" if timing and name in ("xq", "xk", "xv") else "ExternalInput"
        return nc.dram_tensor(name, shape, f32, kind=kind).ap()

    xq = din("xq", [SQ, D])
    xk = din("xk", [SKV, D])
    xv = din("xv", [SKV, D])
    wq, bq = din("wq", [D, D]), din("bq", [1, D])
    wk, bk = din("wk", [D, D]), din("bk", [1, D])
    wv, bv = din("wv", [D, D]), din("bv", [1, D])
    wo, bo = din("wo", [D, D]), din("bo", [1, D])
    out = nc.dram_tensor("out", [SQ, D], f32, kind="ExternalOutput").ap()

    from contextlib import nullcontext

    with tile.TileContext(nc) as tc:
      if timing:
        with tc.tile_pool(name="init", bufs=1) as initp:
            fill = initp.tile([P, D], f32, name="fill")
            nc.vector.memset(fill, 0.01)
            for t_ap, rows in ((xq, SQ), (xk, SKV), (xv, SKV)):
                for rt in range(rows // P):
                    nc.sync.dma_start(t_ap[rt * P : (rt + 1) * P, :], fill)

      with tc.For_i(0, loop, 1) if loop > 1 else nullcontext():
       for rep in range(repeat):
         sx = f"_r{rep}" if repeat > 1 else ""
         st = ExitStack()
         consts = st.enter_context(tc.tile_pool(name=f"consts{sx}", bufs=1))
         onesb = consts.tile([1, D], bf16, name=f"onesb{sx}")
         bo_b = consts.tile([1, D], bf16, name=f"bo_b{sx}")
         bv_b = consts.tile([1, D], bf16, name=f"bv_b{sx}")
         bcol = {}
         for name, ap in [("bq", bq), ("bk", bk)]:
             t = consts.tile([P, FC], f32, tag=f"bcol_{name}", name=f"bc_{name}{sx}")
             nc.sync.dma_start(t, ap.rearrange("o (t p) -> p (o t)", p=P))
             bcol[name] = t
         bvb = consts.tile([P, D], bf16, name=f"bvb{sx}")

         # resident bf16 tensors needed during ingestion
         wT_pool = st.enter_context(tc.tile_pool(name=f"wT{sx}", bufs=1))
         wT = {}
         xT_pool = st.enter_context(tc.tile_pool(name=f"xT{sx}", bufs=1))
         xkT = xT_pool.tile([P, NKT, FC, P], bf16, tag="xkT", name=f"xkT{sx}")
         xvT = xT_pool.tile([P, NKT, FC, P], bf16, tag="xvT", name=f"xvT{sx}")

         # ingestion pools (live for the whole build; tail fits alongside)

         kload = st.enter_context(tc.tile_pool(name=f"kload{sx}", bufs=2))
         kstg_pool = st.enter_context(tc.tile_pool(name=f"kstg{sx}", bufs=2))
         qT_pool = st.enter_context(tc.tile_pool(name=f"QT{sx}", bufs=1))
         QT = [
             qT_pool.tile([P, SQ], bf16, tag=f"QT{dt}", name=f"QT{dt}{sx}")
             for dt in range(FC)
         ]
         # transient prologue staging (closed right after prologue emission)
         accA = st.enter_context(tc.tile_pool(name=f"accA{sx}", bufs=1, space="PSUM"))
         wl_st = ExitStack()
         xqT_pool = wl_st.enter_context(tc.tile_pool(name=f"xqT{sx}", bufs=1))
         wload = wl_st.enter_context(tc.tile_pool(name=f"wload{sx}", bufs=6))
         wloadb = wl_st.enter_context(tc.tile_pool(name=f"wloadb{sx}", bufs=2))
         bst_pool = wl_st.enter_context(tc.tile_pool(name=f"bst{sx}", bufs=1))
         ppsum = wl_st.enter_context(
             tc.tile_pool(name=f"ppsum{sx}", bufs=2, space="PSUM")
         )

         # ---- prologue: DMAs + casts + transposes only (no PSUM) ----
         # x chunk 0 loads go absolutely first so kt0's chain starts early
         x_lds = {}

         def emit_x_load(part, x_ap, sc):
             ld = kload.tile([P, FC, D], f32, tag="kld", name=f"kld{part}_{sc}{sx}")
             x_lds[(part, sc)] = ld
             nc.sync.dma_start(
                 ld,
                 x_ap[sc * 512 : (sc + 1) * 512, :].rearrange(
                     "(p rt) d -> p rt d", p=P
                 ),
             )

         emit_x_load(0, xk, 0)
         emit_x_load(1, xv, 0)

         w_lds = {}
         for wname, w_ap in [("wq", wq), ("wk", wk), ("wv", wv), ("wo", wo)]:
             ld = wload.tile([P, FC, D], f32, tag="wldf", name=f"wldf_{wname}{sx}")
             w_lds[wname] = ld
             nc.sync.dma_start(ld, w_ap.rearrange("(rt p) d -> p rt d", p=P))
         xq_lds = []
         for qh in range(QH):
             ld = wload.tile([P, FC, D], f32, tag="wldf", name=f"xqlf{qh}{sx}")
             nc.sync.dma_start(
                 ld,
                 xq[qh * 512 : (qh + 1) * 512, :].rearrange(
                     "(rt p) d -> p rt d", p=P
                 ),
             )
             xq_lds.append(ld)
         emit_x_load(0, xk, 1)
         emit_x_load(1, xv, 1)

         # bias staging (vector ops, cheap)
         ones_stg = bst_pool.tile([1, D], f32, tag="bstg", name=f"ones_stg{sx}")
         nc.vector.memset(ones_stg, 1.0)
         nc.vector.tensor_copy(onesb, ones_stg)
         stg_bo = bst_pool.tile([1, D], f32, tag="bstg2", name=f"stg_bo{sx}")
         nc.sync.dma_start(stg_bo, bo)
         nc.vector.tensor_copy(bo_b, stg_bo)
         stg_bv = bst_pool.tile([1, D], f32, tag="bstg3", name=f"stg_bv{sx}")
         nc.sync.dma_start(stg_bv, bv)
         nc.vector.tensor_copy(bv_b, stg_bv)

         # chunk 0 casts first on their engines, then w/xq casts+transposes
         kstgs = {}

         def emit_x_cast(part, sc):
             stg = kstg_pool.tile(
                 [P, FC, D], bf16, tag="kstg", name=f"kstg{part}_{sc}{sx}"
             )
             kstgs[(part, sc)] = stg
             nc.vector.tensor_copy(stg, x_lds.pop((part, sc)))

         def emit_x_trans(part, sc):
             xT = xkT if part == 0 else xvT
             nc.sync.dma_start_transpose(
                 xT[:, sc * FC : (sc + 1) * FC, :, :], kstgs.pop((part, sc))
             )

         emit_x_cast(0, 0)
         emit_x_cast(1, 0)

         def emit_wT(wname, pool):
             wt = pool.tile(
                 [P, FC, FC, P], bf16, tag=f"{wname}T", name=f"{wname}T{sx}"
             )
             wT[wname] = wt
             stg = wloadb.tile([P, FC, D], bf16, tag="wld", name=f"wld_{wname}{sx}")
             nc.vector.tensor_copy(stg, w_lds[wname])
             nc.sync.dma_start_transpose(wt, stg)

         emit_x_trans(0, 0)
         emit_x_trans(1, 0)
         emit_wT("wq", wT_pool)
         emit_wT("wk", wT_pool)
         xqT = xqT_pool.tile([P, 2 * FC, FC, P], bf16, name=f"xqT{sx}")
         for qh in range(QH):
             stg = wloadb.tile([P, FC, D], bf16, tag="wld", name=f"xql{qh}{sx}")
             nc.vector.tensor_copy(stg, xq_lds[qh])
             nc.sync.dma_start_transpose(
                 xqT[:, qh * FC : (qh + 1) * FC, :, :], stg
             )
         emit_x_cast(0, 1)
         emit_x_cast(1, 1)
         emit_wT("wv", wT_pool)
         emit_wT("wo", wT_pool)
         emit_x_trans(0, 1)
         emit_x_trans(1, 1)
         for dt in range(FC):
             for qh in range(QH):
                 ps = ppsum.tile([P, 512], f32, tag="qps", name=f"qps{dt}{qh}{sx}")
                 for fc in range(FC):
                     nc.tensor.matmul(
                         ps,
                         lhsT=wT["wq"][:, dt, fc, :],
                         rhs=xqT[:, qh * FC : (qh + 1) * FC, fc, :],
                         start=(fc == 0),
                         stop=(fc == FC - 1),
                     )
                 nc.vector.tensor_scalar_add(
                     QT[dt][:, qh * 512 : (qh + 1) * 512],
                     in0=ps,
                     scalar1=bcol["bq"][:, dt : dt + 1],
                 )
         ps_bvb = ppsum.tile([P, 512], f32, tag="qps", name=f"psbvb{sx}")
         nc.tensor.matmul(
             ps_bvb, lhsT=onesb[:, 0:P], rhs=bv_b, start=True, stop=True
         )
         nc.vector.tensor_copy(bvb, ps_bvb)
         wl_st.close()

         kT_pool = st.enter_context(tc.tile_pool(name=f"KT{sx}", bufs=1))
         KT = [
             kT_pool.tile([P, SKV], bf16, tag=f"KT{dt}", name=f"KT{dt}{sx}")
             for dt in range(FC)
         ]
         vp_pool = st.enter_context(tc.tile_pool(name=f"Vp{sx}", bufs=1))
         Vp = vp_pool.tile([P, NKT, H, DK + 1], bf16, name=f"Vp{sx}")
         attT_pool = st.enter_context(tc.tile_pool(name=f"attT{sx}", bufs=1))
         attT = [
             attT_pool.tile([P, FC, 512], bf16, tag=f"attT{qh}", name=f"attT{qh}{sx}")
             for qh in range(QH)
         ]
         exp_pool = st.enter_context(tc.tile_pool(name=f"exp{sx}", bufs=3))
         norm_pool = st.enter_context(tc.tile_pool(name=f"norm{sx}", bufs=1))
         attU_pool = st.enter_context(tc.tile_pool(name=f"attU{sx}", bufs=2))

         def stash_acc(p, qh, acc):
             """Boundary: one bf16 copy per head (incl denom row); frees the
             acc PSUM banks immediately, normalize happens a half later."""
             us = []
             for i in range(2):
                 u = attU_pool.tile(
                     [DK + 1, 512], bf16, tag=f"attU{i}", name=f"aU{p}_{qh}_{i}{sx}"
                 )
                 nc.vector.tensor_copy(u, acc[i])
                 us.append(u)
             return us

         def late_normalize(p, qh, us):
             rb = norm_pool.tile(
                 [DK, 2, 512], bf16, tag="rb", name=f"rb{p}_{qh}{sx}"
             )
             for i in range(2):
                 rc = norm_pool.tile(
                     [1, 512], bf16, tag="rc", name=f"rc{p}_{qh}_{i}{sx}"
                 )
                 with nc.allow_low_precision(reason="softmax denom recip"):
                     nc.vector.reciprocal(rc, us[i][DK : DK + 1, :])
                 nc.gpsimd.partition_broadcast(rb[:, i, :], rc)
                 nc.vector.tensor_mul(
                     attT[qh][i * DK : (i + 1) * DK, p, :],
                     us[i][0:DK, :],
                     rb[:, i, :],
                 )

         # ---- attention: pair-major ----
         pending = []
         for p in range(H // 2):
             half_st = ExitStack()
             scA = half_st.enter_context(
                 tc.tile_pool(name=f"scA{p}{sx}", bufs=2, space="PSUM")
             )
             pppsum = half_st.enter_context(
                 tc.tile_pool(name=f"pppsum{p}{sx}", bufs=2, space="PSUM")
             )

             def _qtproj_for(dt, qh, pppsum=pppsum):
                 ps = pppsum.tile(
                     [P, 512], f32, tag="pps", name=f"qtp{dt}_{qh}{sx}"
                 )
                 for fc in range(FC):
                     nc.tensor.matmul(
                         ps,
                         lhsT=wT["wq"][:, dt, fc, :],
                         rhs=xqT[:, qh * FC : (qh + 1) * FC, fc, :],
                         start=(fc == 0),
                         stop=(fc == FC - 1),
                     )
                 nc.vector.tensor_scalar_add(
                     QT[dt][:, qh * 512 : (qh + 1) * 512],
                     in0=ps,
                     scalar1=bcol["bq"][:, dt : dt + 1],
                 )

             def _qtproj(qh, p=p):
                 _qtproj_for(p, qh)

             def _bvb(pppsum=pppsum):
                 ps = pppsum.tile([P, 512], f32, tag="pps", name=f"psbvb{sx}")
                 nc.tensor.matmul(
                     ps, lhsT=onesb[:, 0:P], rhs=bv_b, start=True, stop=True
                 )
                 nc.vector.tensor_copy(bvb, ps)

             def _vproj(sc, vt, p=p, pppsum=pppsum):
                 kt = sc * 4 + vt
                 ps = pppsum.tile(
                     [P, 2 * DK], f32, tag="pps", name=f"vps{p}_{sc}_{vt}{sx}"
                 )
                 for fc in range(FC):
                     nc.tensor.matmul(
                         ps,
                         lhsT=xvT[:, kt, fc, :],
                         rhs=wT["wv"][:, p, fc, :],
                         start=(fc == 0),
                         stop=(fc == FC - 1),
                     )
                 nc.vector.tensor_add(
                     Vp[:, kt, 2 * p : 2 * p + 2, 0:DK],
                     ps.rearrange("p (h d) -> p h d", h=2),
                     bvb[:, p * P : (p + 1) * P].rearrange("p (h d) -> p h d", h=2),
                 )
                 if vt == 3:
                     nc.vector.memset(
                         Vp[:, sc * 4 : (sc + 1) * 4, 2 * p : 2 * p + 2, DK : DK + 1],
                         1.0,
                     )

             def _kproj(sc, dt, pppsum=pppsum):
                 ps = pppsum.tile(
                     [P, 512], f32, tag="pps", name=f"kps{dt}_{sc}{sx}"
                 )
                 for fc in range(FC):
                     nc.tensor.matmul(
                         ps,
                         lhsT=wT["wk"][:, dt, fc, :],
                         rhs=xkT[:, sc * FC : (sc + 1) * FC, fc, :],
                         start=(fc == 0),
                         stop=(fc == FC - 1),
                     )
                 nc.vector.tensor_scalar_add(
                     KT[dt][:, sc * 512 : (sc + 1) * 512],
                     in0=ps,
                     scalar1=bcol["bk"][:, dt : dt + 1],
                 )

             load_q = []
             work_q = []
             for sc in range(NSC):
                 if p == 0:
                     for part, x_ap in [(0, xk), (1, xv)]:
                         if sc >= 2:
                             load_q.append(
                                 lambda part=part, x_ap=x_ap, sc=sc: emit_x_load(
                                     part, x_ap, sc
                                 )
                             )
                             work_q.append(
                                 lambda part=part, sc=sc: emit_x_cast(part, sc)
                             )
                             work_q.append(
                                 lambda part=part, sc=sc: emit_x_trans(part, sc)
                             )
                         if part == 0:
                             work_q.append(lambda sc=sc: _kproj(sc, 0))
                         else:
                             for vt in range(4):
                                 work_q.append(
                                     lambda sc=sc, vt=vt: _vproj(sc, vt)
                                 )
                 else:
                     work_q.append(lambda sc=sc, p=p: _kproj(sc, p))
                     for vt in range(4):
                         work_q.append(lambda sc=sc, vt=vt: _vproj(sc, vt))
             wper = (len(work_q) + NSC - 1) // NSC
             lpos = wpos = 0
             npre = 8 if p == 0 else 5
             while wpos < npre:
                 work_q[wpos]()
                 wpos += 1

             # ---- qh = 0 ----
             acc = [
                 accA.tile(
                     [DK + 1, 512], f32, tag=f"acc{i}", name=f"accA{p}_{i}{sx}"
                 )
                 for i in range(2)
             ]
             for kt in range(NKT):
                 ks = slice(kt * P, (kt + 1) * P)
                 sc_ps = scA.tile(
                     [P, 2, 512], f32, tag="sc", name=f"sc{p}_0_{kt}{sx}"
                 )
                 for i in range(2):
                     nc.tensor.matmul(
                         sc_ps[:, i, :],
                         lhsT=KT[p][i * DK : (i + 1) * DK, ks],
                         rhs=QT[p][i * DK : (i + 1) * DK, 0:512],
                         start=True,
                         stop=True,
                     )
                 ex = exp_pool.tile(
                     [P, 2, 512], bf16, tag="ex", name=f"ex{p}_0_{kt}{sx}"
                 )
                 nc.scalar.activation(ex, sc_ps, func=EXP, scale=INV_SCALE)
                 if kt == 4 and pending:
                     late_normalize(*pending.pop(0))
                 # linear pacing; loads lead the work that consumes them
                 wtarget = min(len(work_q), npre + (wper * (kt + 4)) // 4)
                 ltarget = min(len(load_q), 2 * ((wtarget - npre) // wper + 2))
                 while lpos < ltarget:
                     load_q[lpos]()
                     lpos += 1
                 while wpos < wtarget:
                     work_q[wpos]()
                     wpos += 1
                 for i in range(2):
                     nc.tensor.matmul(
                         acc[i],
                         lhsT=Vp[:, kt, 2 * p + i, :],
                         rhs=ex[:, i, :],
                         start=(kt == 0),
                         stop=(kt == NKT - 1),
                     )
             while wpos < len(work_q):
                 work_q[wpos]()
                 wpos += 1
             us0 = stash_acc(p, 0, acc)
             half_st.close()
             pending.append((p, 0, us0))

             # ---- qh = 1 ----
             half_st = ExitStack()
             scB = half_st.enter_context(
                 tc.tile_pool(name=f"scB{p}{sx}", bufs=2, space="PSUM")
             )
             accB = half_st.enter_context(
                 tc.tile_pool(name=f"accB{p}{sx}", bufs=1, space="PSUM")
             )
             acc = [
                 accB.tile(
                     [DK + 1, 512], f32, tag=f"acc{i}", name=f"accB{p}_{i}{sx}"
                 )
                 for i in range(2)
             ]
             for kt in range(NKT):
                 ks = slice(kt * P, (kt + 1) * P)
                 sc_ps = scB.tile(
                     [P, 2, 512], f32, tag="sc", name=f"sc{p}_1_{kt}{sx}"
                 )
                 for i in range(2):
                     nc.tensor.matmul(
                         sc_ps[:, i, :],
                         lhsT=KT[p][i * DK : (i + 1) * DK, ks],
                         rhs=QT[p][i * DK : (i + 1) * DK, 512:1024],
                         start=True,
                         stop=True,
                     )
                 ex = exp_pool.tile(
                     [P, 2, 512], bf16, tag="ex", name=f"ex{p}_1_{kt}{sx}"
                 )
                 nc.scalar.activation(ex, sc_ps, func=EXP, scale=INV_SCALE)
                 if kt == 4 and pending:
                     late_normalize(*pending.pop(0))
                 for i in range(2):
                     nc.tensor.matmul(
                         acc[i],
                         lhsT=Vp[:, kt, 2 * p + i, :],
                         rhs=ex[:, i, :],
                         start=(kt == 0),
                         stop=(kt == NKT - 1),
                     )
             pending.append((p, 1, stash_acc(p, 1, acc)))
             half_st.close()

         # ---- tail: out-proj qh0, last late-normalize, out-proj qh1 ----
         with (
             tc.tile_pool(name=f"opsum{sx}", bufs=4, space="PSUM") as opsum,
             tc.tile_pool(name=f"outbuf{sx}", bufs=2) as outbuf,
         ):

             def oproj(qh):
                 for qt in range(4):
                     po = opsum.tile([P, D], f32, tag="po", name=f"po{qh}_{qt}{sx}")
                     for dt in range(FC):
                         nc.tensor.matmul(
                             po,
                             lhsT=attT[qh][:, dt, qt * P : (qt + 1) * P],
                             rhs=wT["wo"][:, :, dt, :],
                             start=(dt == 0),
                             stop=False,
                         )
                     nc.tensor.matmul(
                         po, lhsT=onesb[:, 0:P], rhs=bo_b, start=False, stop=True
                     )
                     ot = outbuf.tile([P, D], f32, tag="ot", name=f"ot{qh}_{qt}{sx}")
                     nc.vector.tensor_copy(ot, po)
                     nc.sync.dma_start(
                         out[qh * 512 + qt * P : qh * 512 + (qt + 1) * P, :], ot
                     )

             oproj(0)
             while pending:
                 late_normalize(*pending.pop(0))
             oproj(1)
         st.close()

    nc.compile()
    return nc


def get_nc(repeat: int = 1, timing: bool = False, loop: int = 1):
    key = f"nc{repeat}{'t' if timing else ''}l{loop}"
    if key not in _CACHE:
        _CACHE[key] = _build_nc(repeat, timing, loop)
    return _CACHE[key]


def make_in_maps(query, key, value, w_q, b_q, w_k, b_k, w_v, b_v, w_o, b_o):
    query = np.ascontiguousarray(np.asarray(query, dtype=np.float32)).reshape(
        B * S, D
    )
    key = np.asarray(key, dtype=np.float32)
    value = np.asarray(value, dtype=np.float32)
    shared = {
        "wq": np.ascontiguousarray(w_q, dtype=np.float32),
        "bq": np.ascontiguousarray(b_q, dtype=np.float32).reshape(1, D),
        "wk": np.ascontiguousarray(w_k, dtype=np.float32),
        "bk": np.ascontiguousarray(b_k, dtype=np.float32).reshape(1, D),
        "wv": np.ascontiguousarray(w_v, dtype=np.float32),
        "bv": np.ascontiguousarray(b_v, dtype=np.float32).reshape(1, D),
        "wo": np.ascontiguousarray(w_o, dtype=np.float32),
        "bo": np.ascontiguousarray(b_o, dtype=np.float32).reshape(1, D),
    }
    in_maps = []
    for c in range(N_CORES):
        b = c // (N_CORES // B)
        r0 = (c % (N_CORES // B)) * SQ
        in_maps.append(
            {
                "xq": query[b * S + r0 : b * S + r0 + SQ, :],
                "xk": np.ascontiguousarray(key[b]),
                "xv": np.ascontiguousarray(value[b]),
                **shared,
            }
        )
    return in_maps


def kernel(query, key, value, w_q, b_q, w_k, b_k, w_v, b_v, w_o, b_o):
    from concourse import bass_utils

    in_maps = make_in_maps(
        query, key, value, w_q, b_q, w_k, b_k, w_v, b_v, w_o, b_o
    )
    nc = get_nc()
    res = bass_utils.run_bass_kernel_spmd(nc, in_maps, core_ids=list(range(N_CORES)))
    out = np.concatenate([res.results[c]["out"] for c in range(N_CORES)], axis=0)
    return out.reshape(B, S, D)


if __name__ == "__main__":
    nc = get_nc()
    print("built ok")
